# revision 59
# baseline (speedup 1.0000x reference)
"""Multi-head causal attention (b=4, t=2048, k=1024, h=16) on 8 Trainium2 cores.

Sharding: core c = (batch b=c//2, head-group g=c%2). Each core computes one
batch x 8 heads; partial outputs (half heads each, and a further k01/k23
output-projection split) are summed on host (bf16 partials; f32 sum).

Per-core kernel. The PE is the bottleneck (~85% busy) and rides the power
envelope (denser schedules trip DVFS half-clock throttling), attention is a
latency chain (ST -> exp on ACT -> PV):
  - Q/K projections run in fp8-e4m3 DoubleRow mode (2 contraction k-tiles
    per pass, half the cycles); weights are host-prescaled by WSCALE=16 to
    stay in fp8 normal range, compensated in the exp scale. The V path
    stays bf16 (fp8 V noise would exceed the accuracy budget). fp8
    DoublePixel for ST is silently dropped by the compiler (measured) - the
    ST/PV 1-col/cycle output/moving-bandwidth floor stands.
  - q-major attention; consecutive k-tile pairs share one 2-bank psum tile
    so ONE exp covers both (20 exp calls/head instead of 40); diagonal-pair
    mask muls are merged into one strided DVE op; a lazy PV queue (depth 12
    on the first head, 4 after - deeper queues throttle) keeps ST/exp
    flowing before V tiles land.
  - input DMA: host pre-arranges all inputs partition-major (x additionally
    chunk-major) so each transfer is 128 fat contiguous rows; queues are
    both row-bound (~9ns/row) and ~150GB/s each, ~300-450GB/s aggregate,
    with a fixed ~8.5us ring-kick + ~5us engine preamble. Issued from
    scalar/sync/gpsimd in consumption-priority order (QK path first).
  - ALL projection fillers live in head-0/1's front list, ordered by DMA
    arrival but meeting emission deadlines (pair-0 chunk n before head-0
    q-chunk n; V tiles before the PV drains that read them). Violating an
    emission deadline is a SILENT RACE: the instruction stream reads a tile
    whose producer is emitted later -> NaN or small corruption.
  - Phase C (out-projection) splits into k01/k23 halves on separate DRAM
    outputs; k01 fills the first head of pair 3, k23 is qc-gated into the
    last head. The last pair processes the ODD head first so the final
    head's normalize writes ot_s directly (no DMA shift on the tail chain);
    the last 4 tiles ship each 512-col half on separate queues as soon as
    its copy lands.
  - normalize: DVE reciprocal_approx_fast, gpsimd partition_broadcast, DVE
    multiply; odd heads shift to partitions 64-127 via sbuf->sbuf DMA
    (gpsimd queue).
  PSUM: proj(2) + st(2x2-bank=4) + otp(2) = 8 banks.
"""
import sys

sys.path.insert(0, "/opt/trn_rl_repo")

import numpy as np
import ml_dtypes

import concourse.bass as bass
import concourse.mybir as mybir
import concourse.tile as tile
from concourse import bacc
from concourse.bass_utils import run_bass_kernel_spmd
from concourse.masks import make_upper_triangular

F32 = mybir.dt.float32
BF16 = mybir.dt.bfloat16
F8 = mybir.dt.float8e4
DR = mybir.MatmulPerfMode.DoubleRow
EXP = mybir.ActivationFunctionType.Exp
WSCALE = 16.0  # host pre-scales Wq/Wk/Wv by this to keep fp8 values normal

B, T, KD, NH, HS = 4, 2048, 1024, 16, 64
NCORES = 8


def build_nc(t=T, dl=512, hl=8, kd=KD):
    nk = kd // 128       # contraction tiles for projections
    mt = t // 128        # t tiles (k-position tiles in attention)
    dt = dl // 128       # local-dim tiles (head pairs)
    nqc = t // 512       # q chunks
    scale = 1.0 / float(np.sqrt(kd)) / (WSCALE * WSCALE)

    nc = bacc.Bacc("TRN2", target_bir_lowering=False, debug=False, num_devices=NCORES)
    # Inputs are host-prearranged partition-major ([128, ktile, cols]) so
    # every DMA moves 128 fat contiguous rows (queue throughput is both
    # descriptor-row-bound ~9ns/row and byte-bound ~167GB/s per queue).
    # x is additionally chunk-major so each 512-col t-chunk is one fat-row DMA.
    xt_d = nc.dram_tensor(
        "xt", [128, t // 512, kd // 128, 512], F8, kind="ExternalInput"
    )
    xtb_d = nc.dram_tensor(
        "xtb", [128, t // 512, kd // 128, 512], BF16, kind="ExternalInput"
    )
    wq_d = nc.dram_tensor("wq", [128, kd // 128, dl], F8, kind="ExternalInput")
    wk_d = nc.dram_tensor("wk", [128, kd // 128, dl], F8, kind="ExternalInput")
    wv_d = nc.dram_tensor("wv", [128, kd // 128, dl], BF16, kind="ExternalInput")
    wo_d = nc.dram_tensor("wo", [128, dl // 128, kd], BF16, kind="ExternalInput")
    out_d = nc.dram_tensor("out", [t, kd], BF16, kind="ExternalOutput")
    out2_d = nc.dram_tensor("out2", [t, kd], BF16, kind="ExternalOutput")

    with tile.TileContext(nc) as tc:
        with (
            tc.tile_pool(name="persist", bufs=1) as pp,
            tc.tile_pool(name="misc", bufs=1) as mp,
            tc.tile_pool(name="pbe", bufs=8) as pbe,
            tc.tile_pool(name="pbm", bufs=4) as pbm,
            tc.tile_pool(name="pco", bufs=3) as pco,
            tc.tile_pool(name="psum", bufs=1, space="PSUM") as psp,
        ):
            qt_s = pp.tile([128, dt, t], BF16)
            kt_s = pp.tile([128, dt, t], BF16)
            v_s = pp.tile([128, mt, hl, 65], BF16)
            ot_s = pp.tile([128, dt, t], BF16)
            xt_s = pp.tile([128, nqc, nk, 512], F8)
            xtb_s = pp.tile([128, nqc, nk, 512], BF16)
            wq_s = pp.tile([128, nk, dl], F8)
            wk_s = pp.tile([128, nk, dl], F8)
            wv_s = pp.tile([128, nk, dl], BF16)
            wo_s = pp.tile([128, dt, kd], BF16)
            mask_f = mp.tile([128, 128], F32)
            mask_t = mp.tile([128, 128], BF16)
            mask_t2 = mp.tile([128, 2, 128], BF16)
            ones_s = mp.tile([1, 64], F32)
            nc.vector.memset(ones_s[:, :], 1.0)
            make_upper_triangular(nc, mask_f[:, :], val=1.0, diag=True)
            nc.vector.tensor_copy(mask_t[:, :], mask_f[:, :])
            nc.vector.tensor_copy(mask_t2[:, 0, :], mask_f[:, :])
            nc.vector.tensor_copy(mask_t2[:, 1, :], mask_f[:, :])
            nc.vector.memset(v_s[:, :, :, 64], 1.0)

            # --------------- input DMA -------------------------------------
            # Few large-row DMAs; issue from engines idle in the prologue
            # (dma_start costs ~0.5-1us of ISSUING-engine time, and scalar is
            # busy with exp almost immediately). Queue order matches the
            # consumption schedule: QK-path inputs first, V-path staged.
            nc.scalar.dma_start(wq_s[:, :, :], wq_d[:, :, :])
            nc.scalar.dma_start(wk_s[:, :, :], wk_d[:, :, :])
            nc.scalar.dma_start(xt_s[:, 2, :, :], xt_d[:, 2, :, :])
            nc.scalar.dma_start(xt_s[:, 3, :, :], xt_d[:, 3, :, :])
            nc.sync.dma_start(xt_s[:, 0, :, :], xt_d[:, 0, :, :])
            nc.sync.dma_start(xt_s[:, 1, :, :], xt_d[:, 1, :, :])
            nc.sync.dma_start(xtb_s[:, 1, :, :], xtb_d[:, 1, :, :])
            nc.sync.dma_start(xtb_s[:, 3, :, :], xtb_d[:, 3, :, :])
            nc.gpsimd.dma_start(wv_s[:, :, :], wv_d[:, :, :])
            nc.gpsimd.dma_start(xtb_s[:, 0, :, :], xtb_d[:, 0, :, :])
            nc.gpsimd.dma_start(xtb_s[:, 2, :, :], xtb_d[:, 2, :, :])
            nc.gpsimd.dma_start(wo_s[:, :, :], wo_d[:, :, :])

            # --------------- filler emitters -------------------------------
            cnt = [0]

            def emit_qk(w_s, o_s, pair, n, on_act=False):
                cols = slice(512 * n, 512 * n + 512)
                ps = psp.tile([128, 512], F32, name=f"pj{cnt[0]}", tag="proj", bufs=2)
                cnt[0] += 1
                for k in range(nk // 2):
                    nc.tensor.matmul(
                        ps[:, :],
                        w_s[:, 2 * k : 2 * k + 2, 128 * pair : 128 * pair + 128],
                        xt_s[:, n, 2 * k : 2 * k + 2, :],
                        start=(k == 0),
                        stop=(k == nk // 2 - 1),
                        perf_mode=DR,
                    )
                if on_act:
                    nc.scalar.copy(o_s[:, pair, cols], ps[:, :])
                else:
                    nc.vector.tensor_copy(o_s[:, pair, cols], ps[:, :])

            def emit_v(m, on_act=False):
                ps = psp.tile([128, 512], F32, name=f"pv{cnt[0]}", tag="proj", bufs=2)
                cnt[0] += 1
                mc = 128 * (m % 4)
                for k in range(nk):
                    nc.tensor.matmul(
                        ps[:, :],
                        xtb_s[:, m // 4, k, mc : mc + 128],
                        wv_s[:, k, :],
                        start=(k == 0),
                        stop=(k == nk - 1),
                    )
                src = ps[:, :].rearrange("p (h d) -> p h d", h=hl)
                if on_act:
                    nc.scalar.copy(v_s[:, m, :, 0:64], src)
                else:
                    nc.vector.tensor_copy(v_s[:, m, :, 0:64], src)

            def emit_c(m, ks, dst_d, eng, fine=False):
                """Half of phase C for t-tile m, contracting head-pairs `ks`."""
                ob = pco.tile([128, kd], BF16, name=f"ob{cnt[0]}", tag="ob")
                cnt[0] += 1
                rows = slice(128 * m, 128 * m + 128)
                for c in range(kd // 512):
                    ps = psp.tile(
                        [128, 512], F32, name=f"pc{cnt[0]}", tag="proj", bufs=2
                    )
                    cnt[0] += 1
                    cs = slice(512 * c, 512 * c + 512)
                    for j, k in enumerate(ks):
                        nc.tensor.matmul(
                            ps[:, :],
                            ot_s[:, k, rows],
                            wo_s[:, k, cs],
                            start=(j == 0),
                            stop=(j == len(ks) - 1),
                        )
                    nc.vector.tensor_copy(ob[:, cs], ps[:, :])
                    if fine:
                        # tail tiles: ship each half as soon as its copy
                        # lands, on separate queues
                        (nc.scalar if c == 0 else nc.sync).dma_start(
                            dst_d[rows, cs], ob[:, cs]
                        )
                if not fine:
                    eng.dma_start(dst_d[rows, :], ob[:, :])

            # --------------- prologue: first Q/K chunk only ----------------
            emit_qk(wq_s, qt_s, 0, 0, on_act=True)
            emit_qk(wk_s, kt_s, 0, 0, on_act=True)

            # --------------- fused attention + filler ----------------------
            def emit_pv(h, ki, qc, a, b, ex, exo, otp):
                """PV for one (ki, qc) unit; ex columns [exo, exo + b - a)."""
                nc.tensor.matmul(
                    otp[0:65, a - 512 * qc : b - 512 * qc],
                    v_s[:, ki, h, :],
                    ex[:, exo : exo + b - a],
                    start=(ki == 0),
                    stop=(ki == 4 * qc + 3),
                )
                if ki != 4 * qc + 3:
                    return False
                mh, ph = h // 2, 64 * (h % 2)
                den = pbm.tile([1, 512], F32, name=f"dn{h}_{qc}", tag="den")
                nc.vector.tensor_copy(den[:, :], otp[64:65, :])
                rec = pbm.tile([1, 512], F32, name=f"rc{h}_{qc}", tag="rec")
                nc.vector.reciprocal_approx_fast(rec[:, :], den[:, :])
                bc = pbm.tile([64, 512], F32, name=f"bc{h}_{qc}", tag="bc")
                nc.gpsimd.partition_broadcast(bc[:, :], rec[0:1, :])
                cols = slice(512 * qc, 512 * qc + 512)
                if ph == 0:
                    nc.vector.tensor_mul(ot_s[0:64, mh, cols], otp[0:64, :], bc[:, :])
                else:
                    sc = pbm.tile([64, 512], BF16, name=f"sc{h}_{qc}", tag="sc")
                    nc.vector.tensor_mul(sc[:, :], otp[0:64, :], bc[:, :])
                    nc.gpsimd.dma_start(ot_s[64:128, mh, cols], sc[:, :])
                return True

            ngrp_head = sum((4 * qc + 4) // 2 for qc in range(nqc))  # 20

            for p in range(dt):
                front = []
                spread = []
                if p == 0:
                    # ALL projection fillers live in head-0/1's front, ordered
                    # by DMA arrival (xt chunks stream in, V-path inputs land
                    # last) while meeting emission deadlines: pair-0 chunk n
                    # before head-0 q-chunk n; V0-3 before the qc2 force-drain.
                    def F(*specs):
                        out = []
                        for s in specs:
                            if s[0] == 'v':
                                out.append(lambda m=s[1]: emit_v(m))
                            else:
                                w, o = (wq_s, qt_s) if s[0] == 'q' else (wk_s, kt_s)
                                out.append(
                                    lambda w=w, o=o, s=s: emit_qk(w, o, s[1], s[2])
                                )
                        return out

                    # Emission deadlines (violating one = silent race → NaN or
                    # corrupt output): pair-0 chunk n before head-0 q-chunk n
                    # (calls 1/5/11); V tiles before the PV drains that read
                    # them (V0-3 by call 5, V4-7 by ~call 9, V8-11 by ~call
                    # 15, V12-15 by call 19). Within that, order by DMA
                    # arrival: xt/wq/wk land first, wv+xtb staged behind.
                    front = F(
                        ('q', 1, 0), ('k', 1, 0),
                        ('q', 0, 1), ('k', 0, 1),
                        ('v', 0), ('v', 1), ('v', 2), ('v', 3),
                        ('q', 0, 2), ('k', 0, 2),
                        ('q', 2, 0), ('k', 2, 0),
                        ('q', 3, 0), ('k', 3, 0),
                        ('v', 4), ('v', 5), ('v', 6), ('v', 7),
                        ('q', 0, 3), ('k', 0, 3),
                        ('q', 1, 1), ('k', 1, 1),
                        ('v', 8), ('v', 9), ('v', 10), ('v', 11),
                        ('q', 2, 1), ('k', 2, 1),
                        ('v', 12), ('v', 13), ('v', 14), ('v', 15),
                        ('q', 3, 1), ('k', 3, 1),
                        ('q', 1, 2), ('k', 1, 2),
                        ('q', 2, 2), ('k', 2, 2),
                        ('q', 3, 2), ('k', 3, 2),
                        ('q', 1, 3), ('k', 1, 3),
                        ('q', 2, 3), ('k', 2, 3),
                        ('q', 3, 3), ('k', 3, 3),
                    )
                if p == 2:
                    # first half of the k01 output phase fills heads 4-5
                    # (pairs 0,1 are complete once p==1 ends, so this is
                    # emission-safe); second half fills the first head of
                    # the last pair
                    spread = [
                        (lambda m=m: emit_c(m, (0, 1), out2_d, nc.gpsimd))
                        for m in range(mt // 2)
                    ]
                if p == dt - 1:
                    spread += [
                        (lambda m=m: emit_c(m, (0, 1), out2_d, nc.gpsimd))
                        for m in range(mt // 2, mt)
                    ]
                fr = [0]
                fi = [0]
                pui = [0]
                npace = ngrp_head if p == dt - 1 else 2 * ngrp_head

                def maybe_fill():
                    pui[0] += 1
                    took = 0
                    while fr[0] < len(front) and took < 2:
                        front[fr[0]]()
                        fr[0] += 1
                        took += 1
                    if took:
                        return
                    want = pui[0] * len(spread) // npace
                    while fi[0] < min(want, len(spread)):
                        spread[fi[0]]()
                        fi[0] += 1

                horder = (2 * p, 2 * p + 1)
                if p == dt - 1:
                    # odd head FIRST in the last pair: the final head is then
                    # the even one, whose normalize writes ot_s directly (no
                    # sbuf->sbuf DMA shift on the end-of-kernel chain)
                    horder = (2 * p + 1, 2 * p)
                for hi, h in enumerate(horder):
                    mh, ph = h // 2, 64 * (h % 2)
                    if p == dt - 1 and hi == 1:
                        while fi[0] < len(spread):
                            spread[fi[0]]()
                            fi[0] += 1
                        spread = []
                        fi[0] = 0
                        pui[0] = 0
                    # deferred-PV depth: deep for the very first head so the
                    # ST/exp stream never blocks on the late-arriving V tiles
                    pv_depth = 12 if (p == 0 and h == 0) else 4
                    pv_pending = []

                    def drain_pv(limit):
                        while len(pv_pending) > limit:
                            args = pv_pending.pop(0)
                            done = emit_pv(*args)
                            if done and p == dt - 1 and hi == 1:
                                dqc = args[2]
                                spread.extend(
                                    (
                                        lambda m=m: emit_c(
                                            m, (2, 3), out_d,
                                            nc.sync if m % 2 else nc.gpsimd,
                                            fine=(m >= mt - 4),
                                        )
                                    )
                                    for m in range(4 * dqc, 4 * dqc + 4)
                                )
                    qcs = list(range(nqc))
                    for qi, qc in enumerate(qcs):
                        # otp bufs=2: before taking this chunk's psum slot
                        # (= the chunk-before-last's), all deferred PVs and
                        # normalize work touching it must be emitted
                        older = set(qcs[: qi - 1]) if qi >= 2 else ()
                        while pv_pending and pv_pending[0][2] in older:
                            drain_pv(len(pv_pending) - 1)
                        otp = psp.tile(
                            [65, 512], F32, name=f"otp{h}_{qc}", tag="ot", bufs=2
                        )
                        for ki0 in range(0, 4 * qc + 4, 2):
                            st = psp.tile(
                                [128, 1024], F32, name=f"st{h}_{ki0}_{qc}",
                                tag="st", bufs=2,
                            )
                            ex = pbe.tile(
                                [128, 1024], BF16, name=f"ex{h}_{ki0}_{qc}",
                                tag="ex",
                            )
                            # place the two units contiguously (no unwritten
                            # psum gap for exp): unit 2 at w1 if both fit in
                            # bank 0, else at the bank-1 boundary
                            b = 512 * qc + 512
                            a1 = max(128 * ki0, 512 * qc)
                            a2 = max(128 * (ki0 + 1), 512 * qc)
                            w1, w2 = b - a1, b - a2
                            o2 = w1 if w1 + w2 <= 512 else 512
                            ws = [(ki0, a1, 0), (ki0 + 1, a2, o2)]
                            for ki, a, off in ws:
                                nc.tensor.matmul(
                                    st[:, off : off + b - a],
                                    kt_s[ph : ph + 64, mh, 128 * ki : 128 * ki + 128],
                                    qt_s[ph : ph + 64, mh, a:b],
                                    start=True,
                                    stop=True,
                                )
                            nc.scalar.activation(
                                ex[:, 0 : o2 + w2], st[:, 0 : o2 + w2],
                                EXP, scale=scale,
                            )
                            if a1 == 128 * ki0:
                                # both units diagonal (always paired); one
                                # strided mul covers both 128-wide blocks
                                exv = ex[:, 0 : 2 * o2].rearrange(
                                    "p (n c) -> p n c", n=2
                                )[:, :, 0:128]
                                nc.vector.tensor_mul(exv, exv, mask_t2[:, :, :])
                            maybe_fill()
                            pv_pending.extend(
                                (h, ki, qc, a, b, ex, off, otp)
                                for ki, a, off in ws
                            )
                            drain_pv(pv_depth)
                    drain_pv(0)
                while fi[0] < len(spread):
                    spread[fi[0]]()
                    fi[0] += 1

    nc.finalize()
    return nc


_NC_CACHE = {}


def _get_nc(key=(T, 512, 8, KD)):
    if key not in _NC_CACHE:
        _NC_CACHE[key] = build_nc(*key)
    return _NC_CACHE[key]


def _pmajor(a):
    """[n*128, cols] -> partition-major [128, n, cols], contiguous."""
    n = a.shape[0] // 128
    return np.ascontiguousarray(a.reshape(n, 128, a.shape[1]).transpose(1, 0, 2))


def _pcmajor(a):
    """[8*128, 4*512] -> [128, chunk 4, ktile 8, 512], contiguous."""
    return np.ascontiguousarray(
        a.reshape(8, 128, 4, 512).transpose(1, 2, 0, 3)
    )


def make_in_maps(x, Wq, Wk, Wv, Wo, dl=512):
    in_maps = []
    for c in range(NCORES):
        b, g = c // 2, c % 2
        rows = slice(dl * g, dl * (g + 1))
        xt = np.ascontiguousarray(x[b].T)
        in_maps.append(
            {
                "xt": _pcmajor(xt.astype(ml_dtypes.float8_e4m3fn)),
                "xtb": _pcmajor(xt.astype(ml_dtypes.bfloat16)),
                "wq": _pmajor(
                    (Wq[rows, :].T * WSCALE).astype(ml_dtypes.float8_e4m3fn)
                ),
                "wk": _pmajor(
                    (Wk[rows, :].T * WSCALE).astype(ml_dtypes.float8_e4m3fn)
                ),
                "wv": _pmajor(Wv[rows, :].T.astype(ml_dtypes.bfloat16)),
                "wo": _pmajor(Wo[:, rows].T.astype(ml_dtypes.bfloat16)),
            }
        )
    return in_maps


def run_spmd(x, Wq, Wk, Wv, Wo, trace=False):
    nc = _get_nc()
    in_maps = make_in_maps(x, Wq, Wk, Wv, Wo)
    res = run_bass_kernel_spmd(nc, in_maps, list(range(NCORES)), trace=trace)
    outs = [
        res.results[c]["out"].astype(np.float32)
        + res.results[c]["out2"].astype(np.float32)
        for c in range(NCORES)
    ]
    final = np.stack([outs[2 * b] + outs[2 * b + 1] for b in range(B)])
    return final.astype(np.float32), res


def kernel(x, Wq, Wk, Wv, Wo):
    x = np.asarray(x, dtype=np.float32)
    Wq = np.asarray(Wq, dtype=np.float32)
    Wk = np.asarray(Wk, dtype=np.float32)
    Wv = np.asarray(Wv, dtype=np.float32)
    Wo = np.asarray(Wo, dtype=np.float32)
    out, _ = run_spmd(x, Wq, Wk, Wv, Wo)
    return out



# revision 60
# speedup vs baseline: 1.2201x; 1.2201x over previous
"""Multi-head causal attention (b=4, t=2048, k=1024, h=16) on 8 Trainium2 cores.

Sharding: core c = (batch b=c//2, head-group g=c%2). Each core computes one
batch x 8 heads; partial outputs (half heads each, and a further k01/k23
output-projection split) are summed on host (bf16 partials; f32 sum).

Per-core kernel. The PE is the bottleneck (~85% busy) and rides the power
envelope (denser schedules trip DVFS half-clock throttling), attention is a
latency chain (ST -> exp on ACT -> PV):
  - Q/K projections run in fp8-e4m3 DoubleRow mode (2 contraction k-tiles
    per pass, half the cycles); weights are host-prescaled by WSCALE=16 to
    stay in fp8 normal range, compensated in the exp scale. The V path
    stays bf16 (fp8 V noise would exceed the accuracy budget). fp8
    DoublePixel for ST is silently dropped by the compiler (measured) - the
    ST/PV 1-col/cycle output/moving-bandwidth floor stands.
  - q-major attention; consecutive k-tile pairs share one 2-bank psum tile
    so ONE exp covers both (20 exp calls/head instead of 40); diagonal-pair
    mask muls are merged into one strided DVE op; a lazy PV queue (depth 12
    on the first head, 4 after - deeper queues throttle) keeps ST/exp
    flowing before V tiles land.
  - input DMA: host pre-arranges all inputs partition-major (x additionally
    chunk-major) so each transfer is 128 fat contiguous rows; queues are
    both row-bound (~9ns/row) and ~150GB/s each, ~300-450GB/s aggregate,
    with a fixed ~8.5us ring-kick + ~5us engine preamble. Issued from
    scalar/sync/gpsimd in consumption-priority order (QK path first).
  - ALL projection fillers live in head-0/1's front list, ordered by DMA
    arrival but meeting emission deadlines (pair-0 chunk n before head-0
    q-chunk n; V tiles before the PV drains that read them). Violating an
    emission deadline is a SILENT RACE: the instruction stream reads a tile
    whose producer is emitted later -> NaN or small corruption.
  - Phase C (out-projection) splits into k01/k23 halves on separate DRAM
    outputs; k01 fills the first head of pair 3, k23 is qc-gated into the
    last head. The last pair processes the ODD head first so the final
    head's normalize writes ot_s directly (no DMA shift on the tail chain);
    the last 4 tiles ship each 512-col half on separate queues as soon as
    its copy lands.
  - normalize: DVE reciprocal_approx_fast, gpsimd partition_broadcast, DVE
    multiply; odd heads shift to partitions 64-127 via sbuf->sbuf DMA
    (gpsimd queue).
  PSUM: proj(2) + st(2x2-bank=4) + otp(2) = 8 banks.
"""
import sys

sys.path.insert(0, "/opt/trn_rl_repo")

import numpy as np
import ml_dtypes

import concourse.bass as bass
import concourse.mybir as mybir
import concourse.tile as tile
from concourse import bacc
from concourse.bass_utils import run_bass_kernel_spmd
from concourse.masks import make_upper_triangular

F32 = mybir.dt.float32
BF16 = mybir.dt.bfloat16
F8 = mybir.dt.float8e4
DR = mybir.MatmulPerfMode.DoubleRow
EXP = mybir.ActivationFunctionType.Exp
WSCALE = 16.0  # host pre-scales Wq/Wk/Wv by this to keep fp8 values normal

B, T, KD, NH, HS = 4, 2048, 1024, 16, 64
NCORES = 8


def build_nc(t=T, dl=512, hl=8, kd=KD):
    nk = kd // 128       # contraction tiles for projections
    mt = t // 128        # t tiles (k-position tiles in attention)
    dt = dl // 128       # local-dim tiles (head pairs)
    nqc = t // 512       # q chunks
    scale = 1.0 / float(np.sqrt(kd)) / (WSCALE * WSCALE)

    nc = bacc.Bacc("TRN2", target_bir_lowering=False, debug=False, num_devices=NCORES)
    # Inputs are host-prearranged partition-major ([128, ktile, cols]) so
    # every DMA moves 128 fat contiguous rows (queue throughput is both
    # descriptor-row-bound ~9ns/row and byte-bound ~167GB/s per queue).
    # x is additionally chunk-major so each 512-col t-chunk is one fat-row DMA.
    xt_d = nc.dram_tensor(
        "xt", [128, t // 512, kd // 128, 512], F8, kind="ExternalInput"
    )
    xtb_d = nc.dram_tensor(
        "xtb", [128, t // 512, kd // 128, 512], BF16, kind="ExternalInput"
    )
    wq_d = nc.dram_tensor("wq", [128, kd // 128, dl], F8, kind="ExternalInput")
    wk_d = nc.dram_tensor("wk", [128, kd // 128, dl], F8, kind="ExternalInput")
    wv_d = nc.dram_tensor("wv", [128, kd // 128, dl], BF16, kind="ExternalInput")
    wo_d = nc.dram_tensor("wo", [128, dl // 128, kd], BF16, kind="ExternalInput")
    out_d = nc.dram_tensor("out", [t, kd], BF16, kind="ExternalOutput")
    out2_d = nc.dram_tensor("out2", [t, kd], BF16, kind="ExternalOutput")

    with tile.TileContext(nc) as tc:
        with (
            tc.tile_pool(name="persist", bufs=1) as pp,
            tc.tile_pool(name="misc", bufs=1) as mp,
            tc.tile_pool(name="pbe", bufs=8) as pbe,
            tc.tile_pool(name="pbm", bufs=4) as pbm,
            tc.tile_pool(name="pco", bufs=3) as pco,
            tc.tile_pool(name="psum", bufs=1, space="PSUM") as psp,
        ):
            qt_s = pp.tile([128, dt, t], BF16)
            kt_s = pp.tile([128, dt, t], BF16)
            v_s = pp.tile([128, mt, hl, 65], BF16)
            ot_s = pp.tile([128, dt, t], BF16)
            xt_s = pp.tile([128, nqc, nk, 512], F8)
            xtb_s = pp.tile([128, nqc, nk, 512], BF16)
            wq_s = pp.tile([128, nk, dl], F8)
            wk_s = pp.tile([128, nk, dl], F8)
            wv_s = pp.tile([128, nk, dl], BF16)
            wo_s = pp.tile([128, dt, kd], BF16)
            mask_f = mp.tile([128, 128], F32)
            mask_t = mp.tile([128, 128], BF16)
            mask_t2 = mp.tile([128, 2, 128], BF16)
            ones_s = mp.tile([1, 64], F32)
            nc.vector.memset(ones_s[:, :], 1.0)
            make_upper_triangular(nc, mask_f[:, :], val=1.0, diag=True)
            nc.vector.tensor_copy(mask_t[:, :], mask_f[:, :])
            nc.vector.tensor_copy(mask_t2[:, 0, :], mask_f[:, :])
            nc.vector.tensor_copy(mask_t2[:, 1, :], mask_f[:, :])
            nc.vector.memset(v_s[:, :, :, 64], 1.0)

            # --------------- input DMA -------------------------------------
            # Few large-row DMAs; issue from engines idle in the prologue
            # (dma_start costs ~0.5-1us of ISSUING-engine time, and scalar is
            # busy with exp almost immediately). Queue order matches the
            # consumption schedule: QK-path inputs first, V-path staged.
            nc.scalar.dma_start(wq_s[:, :, :], wq_d[:, :, :])
            nc.scalar.dma_start(wk_s[:, :, :], wk_d[:, :, :])
            nc.scalar.dma_start(xt_s[:, 2, :, :], xt_d[:, 2, :, :])
            nc.scalar.dma_start(xt_s[:, 3, :, :], xt_d[:, 3, :, :])
            nc.sync.dma_start(xt_s[:, 0, :, :], xt_d[:, 0, :, :])
            nc.sync.dma_start(xt_s[:, 1, :, :], xt_d[:, 1, :, :])
            nc.sync.dma_start(xtb_s[:, 1, :, :], xtb_d[:, 1, :, :])
            nc.sync.dma_start(xtb_s[:, 3, :, :], xtb_d[:, 3, :, :])
            nc.gpsimd.dma_start(wv_s[:, :, :], wv_d[:, :, :])
            nc.gpsimd.dma_start(xtb_s[:, 0, :, :], xtb_d[:, 0, :, :])
            nc.gpsimd.dma_start(xtb_s[:, 2, :, :], xtb_d[:, 2, :, :])
            nc.gpsimd.dma_start(wo_s[:, :, :], wo_d[:, :, :])

            # --------------- filler emitters -------------------------------
            cnt = [0]

            def emit_qk(w_s, o_s, pair, n, on_act=False):
                cols = slice(512 * n, 512 * n + 512)
                ps = psp.tile([128, 512], F32, name=f"pj{cnt[0]}", tag="proj", bufs=2)
                cnt[0] += 1
                for k in range(nk // 2):
                    nc.tensor.matmul(
                        ps[:, :],
                        w_s[:, 2 * k : 2 * k + 2, 128 * pair : 128 * pair + 128],
                        xt_s[:, n, 2 * k : 2 * k + 2, :],
                        start=(k == 0),
                        stop=(k == nk // 2 - 1),
                        perf_mode=DR,
                    )
                if on_act:
                    nc.scalar.copy(o_s[:, pair, cols], ps[:, :])
                else:
                    nc.vector.tensor_copy(o_s[:, pair, cols], ps[:, :])

            def emit_v(m, on_act=False):
                ps = psp.tile([128, 512], F32, name=f"pv{cnt[0]}", tag="proj", bufs=2)
                cnt[0] += 1
                mc = 128 * (m % 4)
                for k in range(nk):
                    nc.tensor.matmul(
                        ps[:, :],
                        xtb_s[:, m // 4, k, mc : mc + 128],
                        wv_s[:, k, :],
                        start=(k == 0),
                        stop=(k == nk - 1),
                    )
                src = ps[:, :].rearrange("p (h d) -> p h d", h=hl)
                if on_act:
                    nc.scalar.copy(v_s[:, m, :, 0:64], src)
                else:
                    nc.vector.tensor_copy(v_s[:, m, :, 0:64], src)

            def emit_c(m, ks, dst_d, eng, fine=False):
                """Half of phase C for t-tile m, contracting head-pairs `ks`."""
                ob = pco.tile([128, kd], BF16, name=f"ob{cnt[0]}", tag="ob")
                cnt[0] += 1
                rows = slice(128 * m, 128 * m + 128)
                for c in range(kd // 512):
                    ps = psp.tile(
                        [128, 512], F32, name=f"pc{cnt[0]}", tag="proj", bufs=2
                    )
                    cnt[0] += 1
                    cs = slice(512 * c, 512 * c + 512)
                    for j, k in enumerate(ks):
                        nc.tensor.matmul(
                            ps[:, :],
                            ot_s[:, k, rows],
                            wo_s[:, k, cs],
                            start=(j == 0),
                            stop=(j == len(ks) - 1),
                        )
                    nc.vector.tensor_copy(ob[:, cs], ps[:, :])
                    if fine:
                        # tail tiles: ship each half as soon as its copy
                        # lands, on separate queues
                        (nc.scalar if c == 0 else nc.sync).dma_start(
                            dst_d[rows, cs], ob[:, cs]
                        )
                if not fine:
                    eng.dma_start(dst_d[rows, :], ob[:, :])

            # --------------- prologue: first Q/K chunk only ----------------
            emit_qk(wq_s, qt_s, 0, 0, on_act=True)
            emit_qk(wk_s, kt_s, 0, 0, on_act=True)

            # --------------- fused attention + filler ----------------------
            def emit_pv(h, ki, qc, a, b, ex, exo, otp):
                """PV for one (ki, qc) unit; ex columns [exo, exo + b - a)."""
                nc.tensor.matmul(
                    otp[0:65, a - 512 * qc : b - 512 * qc],
                    v_s[:, ki, h, :],
                    ex[:, exo : exo + b - a],
                    start=(ki == 0),
                    stop=(ki == 4 * qc + 3),
                )
                if ki != 4 * qc + 3:
                    return False
                mh, ph = h // 2, 64 * (h % 2)
                den = pbm.tile([1, 512], F32, name=f"dn{h}_{qc}", tag="den")
                nc.vector.tensor_copy(den[:, :], otp[64:65, :])
                rec = pbm.tile([1, 512], F32, name=f"rc{h}_{qc}", tag="rec")
                nc.vector.reciprocal_approx_fast(rec[:, :], den[:, :])
                bc = pbm.tile([64, 512], F32, name=f"bc{h}_{qc}", tag="bc")
                nc.gpsimd.partition_broadcast(bc[:, :], rec[0:1, :])
                cols = slice(512 * qc, 512 * qc + 512)
                if ph == 0:
                    nc.vector.tensor_mul(ot_s[0:64, mh, cols], otp[0:64, :], bc[:, :])
                else:
                    sc = pbm.tile([64, 512], BF16, name=f"sc{h}_{qc}", tag="sc")
                    nc.vector.tensor_mul(sc[:, :], otp[0:64, :], bc[:, :])
                    nc.gpsimd.dma_start(ot_s[64:128, mh, cols], sc[:, :])
                return True

            ngrp_head = sum((4 * qc + 4) // 2 for qc in range(nqc))  # 20

            for p in range(dt):
                front = []
                spread = []
                if p == 0:
                    # ALL projection fillers live in head-0/1's front, ordered
                    # by DMA arrival (xt chunks stream in, V-path inputs land
                    # last) while meeting emission deadlines: pair-0 chunk n
                    # before head-0 q-chunk n; V0-3 before the qc2 force-drain.
                    def F(*specs):
                        out = []
                        for s in specs:
                            if s[0] == 'v':
                                out.append(lambda m=s[1]: emit_v(m))
                            else:
                                w, o = (wq_s, qt_s) if s[0] == 'q' else (wk_s, kt_s)
                                out.append(
                                    lambda w=w, o=o, s=s: emit_qk(w, o, s[1], s[2])
                                )
                        return out

                    # Emission deadlines (violating one = silent race → NaN or
                    # corrupt output): pair-0 chunk n before head-0 q-chunk n
                    # (calls 1/5/11); V tiles before the PV drains that read
                    # them (V0-3 by call 5, V4-7 by ~call 9, V8-11 by ~call
                    # 15, V12-15 by call 19). Within that, order by DMA
                    # arrival: xt/wq/wk land first, wv+xtb staged behind.
                    front = F(
                        ('q', 1, 0), ('k', 1, 0),
                        ('q', 0, 1), ('k', 0, 1),
                        ('v', 0), ('v', 1), ('v', 2), ('v', 3),
                        ('q', 0, 2), ('k', 0, 2),
                        ('q', 2, 0), ('k', 2, 0),
                        ('q', 3, 0), ('k', 3, 0),
                        ('v', 4), ('v', 5), ('v', 6), ('v', 7),
                        ('q', 0, 3), ('k', 0, 3),
                        ('q', 1, 1), ('k', 1, 1),
                        ('v', 8), ('v', 9), ('v', 10), ('v', 11),
                        ('q', 2, 1), ('k', 2, 1),
                        ('v', 12), ('v', 13), ('v', 14), ('v', 15),
                        ('q', 3, 1), ('k', 3, 1),
                        ('q', 1, 2), ('k', 1, 2),
                        ('q', 2, 2), ('k', 2, 2),
                        ('q', 3, 2), ('k', 3, 2),
                        ('q', 1, 3), ('k', 1, 3),
                        ('q', 2, 3), ('k', 2, 3),
                        ('q', 3, 3), ('k', 3, 3),
                    )
                if p == dt - 1:
                    spread += [
                        (lambda m=m: emit_c(m, (0, 1), out2_d, nc.gpsimd))
                        for m in range(mt)
                    ]
                fr = [0]
                fi = [0]
                pui = [0]
                npace = ngrp_head if p == dt - 1 else 2 * ngrp_head

                def maybe_fill():
                    pui[0] += 1
                    took = 0
                    while fr[0] < len(front) and took < 2:
                        front[fr[0]]()
                        fr[0] += 1
                        took += 1
                    if took:
                        return
                    want = pui[0] * len(spread) // npace
                    while fi[0] < min(want, len(spread)):
                        spread[fi[0]]()
                        fi[0] += 1

                horder = (2 * p, 2 * p + 1)
                if p == dt - 1:
                    # odd head FIRST in the last pair: the final head is then
                    # the even one, whose normalize writes ot_s directly (no
                    # sbuf->sbuf DMA shift on the end-of-kernel chain)
                    horder = (2 * p + 1, 2 * p)
                for hi, h in enumerate(horder):
                    mh, ph = h // 2, 64 * (h % 2)
                    if p == dt - 1 and hi == 1:
                        while fi[0] < len(spread):
                            spread[fi[0]]()
                            fi[0] += 1
                        spread = []
                        fi[0] = 0
                        pui[0] = 0
                    # deferred-PV depth: deep for the very first head so the
                    # ST/exp stream never blocks on the late-arriving V tiles
                    pv_depth = 12 if (p == 0 and h == 0) else 4
                    pv_pending = []

                    def drain_pv(limit):
                        while len(pv_pending) > limit:
                            args = pv_pending.pop(0)
                            done = emit_pv(*args)
                            if done and p == dt - 1 and hi == 1:
                                dqc = args[2]
                                spread.extend(
                                    (
                                        lambda m=m: emit_c(
                                            m, (2, 3), out_d,
                                            nc.sync if m % 2 else nc.gpsimd,
                                            fine=(m >= mt - 4),
                                        )
                                    )
                                    for m in range(4 * dqc, 4 * dqc + 4)
                                )
                    qcs = list(range(nqc))
                    for qi, qc in enumerate(qcs):
                        # otp bufs=2: before taking this chunk's psum slot
                        # (= the chunk-before-last's), all deferred PVs and
                        # normalize work touching it must be emitted
                        older = set(qcs[: qi - 1]) if qi >= 2 else ()
                        while pv_pending and pv_pending[0][2] in older:
                            drain_pv(len(pv_pending) - 1)
                        otp = psp.tile(
                            [65, 512], F32, name=f"otp{h}_{qc}", tag="ot", bufs=2
                        )
                        for ki0 in range(0, 4 * qc + 4, 2):
                            st = psp.tile(
                                [128, 1024], F32, name=f"st{h}_{ki0}_{qc}",
                                tag="st", bufs=2,
                            )
                            ex = pbe.tile(
                                [128, 1024], BF16, name=f"ex{h}_{ki0}_{qc}",
                                tag="ex",
                            )
                            # place the two units contiguously (no unwritten
                            # psum gap for exp): unit 2 at w1 if both fit in
                            # bank 0, else at the bank-1 boundary
                            b = 512 * qc + 512
                            a1 = max(128 * ki0, 512 * qc)
                            a2 = max(128 * (ki0 + 1), 512 * qc)
                            w1, w2 = b - a1, b - a2
                            o2 = w1 if w1 + w2 <= 512 else 512
                            ws = [(ki0, a1, 0), (ki0 + 1, a2, o2)]
                            for ki, a, off in ws:
                                nc.tensor.matmul(
                                    st[:, off : off + b - a],
                                    kt_s[ph : ph + 64, mh, 128 * ki : 128 * ki + 128],
                                    qt_s[ph : ph + 64, mh, a:b],
                                    start=True,
                                    stop=True,
                                )
                            nc.scalar.activation(
                                ex[:, 0 : o2 + w2], st[:, 0 : o2 + w2],
                                EXP, scale=scale,
                            )
                            if a1 == 128 * ki0:
                                # both units diagonal (always paired); one
                                # strided mul covers both 128-wide blocks
                                exv = ex[:, 0 : 2 * o2].rearrange(
                                    "p (n c) -> p n c", n=2
                                )[:, :, 0:128]
                                nc.vector.tensor_mul(exv, exv, mask_t2[:, :, :])
                            maybe_fill()
                            pv_pending.extend(
                                (h, ki, qc, a, b, ex, off, otp)
                                for ki, a, off in ws
                            )
                            drain_pv(pv_depth)
                    drain_pv(0)
                while fi[0] < len(spread):
                    spread[fi[0]]()
                    fi[0] += 1

    nc.finalize()
    return nc


_NC_CACHE = {}


def _get_nc(key=(T, 512, 8, KD)):
    if key not in _NC_CACHE:
        _NC_CACHE[key] = build_nc(*key)
    return _NC_CACHE[key]


def _pmajor(a):
    """[n*128, cols] -> partition-major [128, n, cols], contiguous."""
    n = a.shape[0] // 128
    return np.ascontiguousarray(a.reshape(n, 128, a.shape[1]).transpose(1, 0, 2))


def _pcmajor(a):
    """[8*128, 4*512] -> [128, chunk 4, ktile 8, 512], contiguous."""
    return np.ascontiguousarray(
        a.reshape(8, 128, 4, 512).transpose(1, 2, 0, 3)
    )


def make_in_maps(x, Wq, Wk, Wv, Wo, dl=512):
    in_maps = []
    for c in range(NCORES):
        b, g = c // 2, c % 2
        rows = slice(dl * g, dl * (g + 1))
        xt = np.ascontiguousarray(x[b].T)
        in_maps.append(
            {
                "xt": _pcmajor(xt.astype(ml_dtypes.float8_e4m3fn)),
                "xtb": _pcmajor(xt.astype(ml_dtypes.bfloat16)),
                "wq": _pmajor(
                    (Wq[rows, :].T * WSCALE).astype(ml_dtypes.float8_e4m3fn)
                ),
                "wk": _pmajor(
                    (Wk[rows, :].T * WSCALE).astype(ml_dtypes.float8_e4m3fn)
                ),
                "wv": _pmajor(Wv[rows, :].T.astype(ml_dtypes.bfloat16)),
                "wo": _pmajor(Wo[:, rows].T.astype(ml_dtypes.bfloat16)),
            }
        )
    return in_maps


def run_spmd(x, Wq, Wk, Wv, Wo, trace=False):
    nc = _get_nc()
    in_maps = make_in_maps(x, Wq, Wk, Wv, Wo)
    res = run_bass_kernel_spmd(nc, in_maps, list(range(NCORES)), trace=trace)
    outs = [
        res.results[c]["out"].astype(np.float32)
        + res.results[c]["out2"].astype(np.float32)
        for c in range(NCORES)
    ]
    final = np.stack([outs[2 * b] + outs[2 * b + 1] for b in range(B)])
    return final.astype(np.float32), res


def kernel(x, Wq, Wk, Wv, Wo):
    x = np.asarray(x, dtype=np.float32)
    Wq = np.asarray(Wq, dtype=np.float32)
    Wk = np.asarray(Wk, dtype=np.float32)
    Wv = np.asarray(Wv, dtype=np.float32)
    Wo = np.asarray(Wo, dtype=np.float32)
    out, _ = run_spmd(x, Wq, Wk, Wv, Wo)
    return out

